# revision 1
# baseline (speedup 1.0000x reference)
"""Multi-head attention (B=2, S=2048, D=1024, H=16) on 8 TRN2 NeuronCores.

Sharding: tensor parallel over heads (2 heads/core) for QKV projection +
attention, then an AllToAll of the context (channel-shard -> row-shard),
then row-parallel output projection. Inputs arrive full; sharding happens
host-side in `kernel()`.

Matmuls run in bf16 (fp32r leaves the PE clock-gated cold and costs 1.5
cyc/row; bf16 is 1 cyc/row, warms HAM, and gets fast weight loads).
Softmax statistics stay fp32 in PSUM; 1/sum is computed as exp(-ln(s)) on
the Scalar engine so the Vector engine never blocks the PE pipeline.

The attention q-range is strided across cores so each of the two AllToAll
halves carries a fully-populated buffer, letting collective #1 and the
first half of the output projection overlap the second attention pass.

Self-contained: shapes hardcoded, no sibling imports.
"""

import numpy as np

B, S, D, H = 2, 2048, 1024, 16
NCORES = 8
CH = D // NCORES          # 128 channels (2 heads) per core
HD = D // H               # 64
ROWS = B * S              # 4096
RPC = ROWS // NCORES      # 512 rows per core for the output projection
KO = D // 128             # 8 contraction chunks of 128
QCH = 1024                # q-chunk processed per attention pass
NQ = S // QCH             # 2 passes
KB = S // 128             # 16 key blocks
RH = RPC // NQ            # 256 rows per core per A2A half
SCALE = 1.0 / 32.0        # 1/sqrt(D)

_CACHE = {}


def _build():
    import concourse.mybir as mybir
    import concourse.tile as tile
    from concourse import bacc
    from concourse.masks import make_identity

    BF16 = mybir.dt.bfloat16
    F32 = mybir.dt.float32
    AF = mybir.ActivationFunctionType

    nc = bacc.Bacc("TRN2", target_bir_lowering=False, debug=False, num_devices=NCORES)
    xT = nc.dram_tensor("xT", [D, ROWS], BF16, kind="ExternalInput")
    # weights arrive host-pre-tiled as [128, KO, out] so DMAs are contiguous
    wq = nc.dram_tensor("wq", [128, KO, CH], BF16, kind="ExternalInput")
    wk = nc.dram_tensor("wk", [128, KO, CH], BF16, kind="ExternalInput")
    wv = nc.dram_tensor("wv", [128, KO, CH], BF16, kind="ExternalInput")
    wo = nc.dram_tensor("wo", [128, KO, D], BF16, kind="ExternalInput")
    out = nc.dram_tensor("out", [RPC, D], F32, kind="ExternalOutput")

    with tile.TileContext(nc) as tc:
        with (
            tc.tile_pool(name="const", bufs=1) as cpool,
            tc.tile_pool(name="qkv", bufs=16) as qkvp,
            tc.tile_pool(name="vt", bufs=3) as vtp,
            tc.tile_pool(name="vtr", bufs=8) as vtrp,
            tc.tile_pool(name="xt", bufs=6) as xtp,
            tc.tile_pool(name="exp", bufs=6) as expp,
            tc.tile_pool(name="bc", bufs=2) as bcp,
            tc.tile_pool(name="cs", bufs=2) as csp,
            tc.tile_pool(name="ph2", bufs=2) as ph2p,
            tc.tile_pool(name="osb", bufs=2) as osbp,
            tc.tile_pool(name="ps", bufs=2, space="PSUM") as ps,
            tc.tile_pool(name="dram", bufs=1, space="DRAM") as dram,
        ):
            w_tiles = {}
            for name, t in (("wq", wq), ("wk", wk), ("wv", wv)):
                wt = cpool.tile([128, KO, CH], BF16, tag=name)
                nc.sync.dma_start(wt[:], t[:])
                w_tiles[name] = wt
            ident = cpool.tile([128, 128], BF16, tag="ident")
            make_identity(nc, ident[:])

            a2a_in = [dram.tile([NCORES, CH, RH], BF16, name=f"a2a_in{p}") for p in range(NQ)]
            a2a_out = [dram.tile([NCORES, CH, RH], BF16, name=f"a2a_out{p}") for p in range(NQ)]

            xT_r = xT.ap().rearrange("(ko p) n -> p ko n", p=128)
            NRB = S // 512  # rowblocks per batch

            qts = {0: [None] * NQ, 1: [None] * NQ}
            kts = {0: [None] * NRB, 1: [None] * NRB}
            vrs = {0: [None] * NRB, 1: [None] * NRB}

            def proj_rowblock(b, rb):
                """project one 512-row block; V transposed into row-major
                [krows, ch] blocks with a fused ones column per head."""
                r = b * NRB + rb
                xt = xtp.tile([128, KO, 512], BF16, tag="xt")
                nc.sync.dma_start(xt[:], xT_r[:, :, r * 512:(r + 1) * 512])
                # q is stored per head, padded with a zeroed half, so the
                # scores matmul can contract over the full 128 partitions --
                # same cycle count, but the PE array streams full-width (keeps
                # the HAM clock gate warm)
                if rb % 2 == 0:
                    qp = [
                        qkvp.tile([128, QCH], BF16, tag="qt",
                                  name=f"qt{b}_{rb // 2}_{h}")
                        for h in range(2)
                    ]
                    nc.vector.memset(qp[0][64:128, :], 0.0)
                    nc.vector.memset(qp[1][0:64, :], 0.0)
                    qts[b][rb // 2] = qp
                qt = qts[b][rb // 2]
                qoff = (rb % 2) * 512
                kt = qkvp.tile([128, 512], BF16, tag="kt", name=f"kt{b}_{rb}")
                vt = vtp.tile([128, 512], BF16, tag="vt")
                for wname, dst in (("wq", None), ("wk", kt), ("wv", vt)):
                    pj = ps.tile([128, 512], F32, tag="sc")
                    for ko in range(KO):
                        nc.tensor.matmul(
                            pj[:], w_tiles[wname][:, ko, :], xt[:, ko, :],
                            start=(ko == 0), stop=(ko == KO - 1),
                        )
                    if wname == "wq":
                        # each head's q lives on the partitions matching its
                        # own channels in kt; the other half stays zero
                        nc.vector.tensor_copy(
                            qt[0][0:64, qoff:qoff + 512], pj[0:64, :])
                        nc.vector.tensor_copy(
                            qt[1][64:128, qoff:qoff + 512], pj[64:128, :])
                    else:
                        nc.vector.tensor_copy(dst[:], pj[:])
                # vr: per head a full-128 lhsT block [V_h | 1 | zeros]
                vr = vtrp.tile([128, 4, 256], BF16, tag="vtr", name=f"vr{b}_{rb}")
                nc.vector.memset(vr[:], 0.0)
                nc.vector.memset(vr[:, :, 64:65], 1.0)
                nc.vector.memset(vr[:, :, 192:193], 1.0)
                for j in range(4):
                    tp = ps.tile([128, 128], BF16, tag="sc", name=f"tp{b}_{rb}_{j}")
                    nc.tensor.transpose(tp[:], vt[:, j * 128:(j + 1) * 128], ident[:])
                    nc.vector.tensor_copy(vr[:, j, 0:64], tp[:, 0:64])
                    nc.vector.tensor_copy(vr[:, j, 128:192], tp[:, 64:128])
                kts[b][rb], vrs[b][rb] = kt, vr

            def attn_chunk(b, p, ctx_ps, kb_range):
                for kb in kb_range:
                    krb, kj = kb // 4, kb % 4
                    scs = [
                        ps.tile([128, QCH], F32, tag="sc", name=f"sc_{b}_{p}_{kb}_{h}")
                        for h in range(2)
                    ]
                    # full-partition contraction: the upper half of each
                    # padded q tile is zero, so the other head's k rows
                    # contribute nothing
                    for h in range(2):
                        for n in range(QCH // 512):
                            nc.tensor.matmul(
                                scs[h][:, n * 512:(n + 1) * 512],
                                kts[b][krb][:, kj * 128:(kj + 1) * 128],
                                qts[b][p][h][:, n * 512:(n + 1) * 512],
                                start=True, stop=True,
                            )
                    ex = []
                    for h in range(2):
                        e = expp.tile([128, QCH], BF16, tag="exp")
                        nc.scalar.activation(e[:], scs[h][:], AF.Exp, scale=SCALE)
                        ex.append(e)
                    for h in range(2):
                        for n in range(QCH // 512):
                            nc.tensor.matmul(
                                ctx_ps[h][:, n * 512:(n + 1) * 512],
                                vrs[b][krb][:, kj, h * 128:(h + 1) * 128],
                                ex[h][:, n * 512:(n + 1) * 512],
                                start=(kb == 0), stop=(kb == KB - 1),
                            )

            def free_ctx(b, p, ctx_ps):
                # fast DVE copies release the ctx psum slots for the next pass
                cfs = []
                for h in range(2):
                    cf = csp.tile([65, QCH], F32, tag="cf", name=f"cf_{b}_{p}_{h}")
                    nc.vector.tensor_copy(cf[:], ctx_ps[h][0:65, :])
                    cfs.append(cf)
                return cfs

            def ship(b, p, cfs, last=False):
                # normalize from SBUF and scatter into the A2A buffer; emitted
                # after any later input-prefetch DMAs so those aren't queued
                # behind these writes
                for h in range(2):
                    cf = cfs[h]
                    bc = bcp.tile([64, QCH], F32, tag="bc")
                    if last:
                        # tail: ACT's ln+exp reciprocal is ~3x faster than
                        # DVE's exact reciprocal and ACT is idle by now
                        lt = bcp.tile([1, QCH], F32, tag="lt")
                        nc.scalar.activation(lt[:], cf[64:65, :], AF.Ln)
                        nc.scalar.activation(bc[0:1, :], lt[:], AF.Exp, scale=-1.0)
                    else:
                        nc.vector.reciprocal(bc[0:1, :], cf[64:65, :])
                    nc.gpsimd.partition_broadcast(bc[:], bc[0:1, :], channels=64)
                    cs = csp.tile([64, QCH], BF16, tag="cs")
                    nc.vector.tensor_mul(cs[:], cf[0:64, :], bc[:])
                    # q within the pass decomposes as (v, j, i) -> dst core
                    # 4b+j, local row v*128+i
                    nc.sync.dma_start(
                        a2a_in[p][4 * b:4 * b + 4, h * 64:(h + 1) * 64, :]
                        .rearrange("j c (v i) -> c v j i", i=128),
                        cs[:].rearrange("c (v j i) -> c v j i", v=2, j=4),
                    )

            def phase2_half(p, wo_t):
                ctxg = ph2p.tile([128, KO, RH], BF16, tag="ctxg", name=f"ctxg{p}")
                nc.sync.dma_start(ctxg[:], a2a_out[p][:].rearrange("j q r -> q j r"))
                for rb in range(RH // 128):
                    for nh in range(D // 512):
                        pj = ps.tile(
                            [128, 512], F32,
                            tag=("cx" if (rb + nh) % 2 else "sc"),
                            name=f"p2_{p}_{rb}_{nh}",
                        )
                        for j in range(KO):
                            nc.tensor.matmul(
                                pj[:],
                                ctxg[:, j, rb * 128:(rb + 1) * 128],
                                wo_t[:, j, nh * 512:(nh + 1) * 512],
                                start=(j == 0), stop=(j == KO - 1),
                            )
                        ob = osbp.tile([128, 512], F32, tag="osb")
                        nc.vector.tensor_copy(ob[:], pj[:])
                        nc.sync.dma_start(
                            out.ap()[p * RH + rb * 128:p * RH + (rb + 1) * 128,
                                     nh * 512:(nh + 1) * 512],
                            ob[:],
                        )

            def ctx_alloc(b, p):
                return [
                    ps.tile([128, QCH], F32, tag="cx", name=f"ctx_{b}_{p}_{h}")
                    for h in range(2)
                ]

            # pass p=0 pipelines the projection inside the attention kblk
            # loop (ScalarE starts ~50us earlier); proj psum shares the "sc"
            # FIFO with scores in emission order
            proj_rowblock(0, 0)
            proj_rowblock(0, 1)
            ctx00 = ctx_alloc(0, 0)
            attn_chunk(0, 0, ctx00, range(0, 4))
            proj_rowblock(0, 2)
            attn_chunk(0, 0, ctx00, range(4, 8))
            proj_rowblock(0, 3)
            attn_chunk(0, 0, ctx00, range(8, 16))
            cfs00 = free_ctx(0, 0, ctx00)
            proj_rowblock(1, 0)
            proj_rowblock(1, 1)
            ctx10 = ctx_alloc(1, 0)
            attn_chunk(1, 0, ctx10, range(0, 4))
            proj_rowblock(1, 2)
            attn_chunk(1, 0, ctx10, range(4, 8))
            proj_rowblock(1, 3)
            # ship after b1's proj so its slow reciprocal doesn't sit ahead of
            # the proj psum-freeing copies in the in-order DVE queue
            ship(0, 0, cfs00)
            wo_t = cpool.tile([128, KO, D], BF16, tag="wo")
            nc.sync.dma_start(wo_t[:], wo[:])
            attn_chunk(1, 0, ctx10, range(8, 16))
            cfs10 = free_ctx(1, 0, ctx10)
            ship(1, 0, cfs10)
            nc.gpsimd.collective_compute(
                "AllToAll", mybir.AluOpType.bypass,
                replica_groups=[list(range(NCORES))],
                ins=[a2a_in[0].opt()], outs=[a2a_out[0].opt()],
            )
            for b in range(B):
                ctx = ctx_alloc(b, 1)
                attn_chunk(b, 1, ctx, range(KB))
                cfs = free_ctx(b, 1, ctx)
                ship(b, 1, cfs, last=(b == 1))
            # emitted before the collective: Tile orders post-collective work
            # after it, so half 0 (whose data arrived with collective #0)
            # must precede to fill the skew window while A2A#1 completes
            phase2_half(0, wo_t)
            nc.gpsimd.collective_compute(
                "AllToAll", mybir.AluOpType.bypass,
                replica_groups=[list(range(NCORES))],
                ins=[a2a_in[1].opt()], outs=[a2a_out[1].opt()],
            )
            phase2_half(1, wo_t)
    nc.compile()
    return nc


def _numpy_reference(tensor_in, attention_mask, Wq, Wk, Wv, Wo):
    """Fallback for a non-zero mask (never hit with the spec's zero mask)."""
    x = tensor_in.astype(np.float64)
    q = (x @ Wq.T.astype(np.float64)).reshape(B, S, H, HD).transpose(0, 2, 1, 3)
    k = (x @ Wk.T.astype(np.float64)).reshape(B, S, H, HD).transpose(0, 2, 1, 3)
    v = (x @ Wv.T.astype(np.float64)).reshape(B, S, H, HD).transpose(0, 2, 1, 3)
    scores = np.einsum("bhqd,bhkd->bhqk", q, k) + attention_mask.astype(np.float64)
    scores = scores / np.sqrt(D)
    scores -= scores.max(axis=-1, keepdims=True)
    w = np.exp(scores)
    w /= w.sum(axis=-1, keepdims=True)
    ctx = np.einsum("bhqk,bhkd->bhqd", w, v).transpose(0, 2, 1, 3).reshape(B, S, D)
    return (ctx @ Wo.T.astype(np.float64)).astype(np.float32)


def _pretile(wT: np.ndarray) -> np.ndarray:
    """[D, M] -> [128, KO, M] with row d = ko*128 + p."""
    m = wT.shape[1]
    return np.ascontiguousarray(wT.reshape(KO, 128, m).transpose(1, 0, 2))


def _row_map() -> np.ndarray:
    """global row index handled by (core c, local row lr)."""
    m = np.empty((NCORES, RPC), dtype=np.int64)
    for c in range(NCORES):
        bb, jj = c // 4, c % 4
        for p in range(NQ):
            for rb in range(RH // 128):
                u = 2 * p + rb
                g = bb * S + jj * 128 + 512 * u
                lr = p * RH + rb * 128
                m[c, lr:lr + 128] = np.arange(g, g + 128)
    return m


def _run(inputs, trace=False):
    import ml_dtypes
    from concourse.bass_utils import run_bass_kernel_spmd

    bf16 = ml_dtypes.bfloat16
    tensor_in = np.asarray(inputs["tensor_in"], dtype=np.float32)
    Wq = np.asarray(inputs["Wq"], dtype=np.float32)
    Wk = np.asarray(inputs["Wk"], dtype=np.float32)
    Wv = np.asarray(inputs["Wv"], dtype=np.float32)
    Wo = np.asarray(inputs["Wo"], dtype=np.float32)

    xT = np.ascontiguousarray(tensor_in.reshape(ROWS, D).T).astype(bf16)
    wqT = Wq.T.astype(bf16)
    wkT = Wk.T.astype(bf16)
    wvT = Wv.T.astype(bf16)
    wo_p = _pretile(Wo.T.astype(bf16))

    in_maps = []
    for c in range(NCORES):
        sl = slice(c * CH, (c + 1) * CH)
        in_maps.append({
            "xT": xT,
            "wq": _pretile(wqT[:, sl]),
            "wk": _pretile(wkT[:, sl]),
            "wv": _pretile(wvT[:, sl]),
            "wo": wo_p,
        })

    if "nc" not in _CACHE:
        _CACHE["nc"] = _build()
    res = run_bass_kernel_spmd(
        _CACHE["nc"], in_maps, core_ids=list(range(NCORES)), trace=trace
    )
    rm = _CACHE.setdefault("rm", _row_map())
    full = np.empty((ROWS, D), dtype=np.float32)
    for c in range(NCORES):
        full[rm[c]] = res.results[c]["out"]
    return full.reshape(B, S, D), res


def kernel(**inputs) -> np.ndarray:
    mask = np.asarray(inputs["attention_mask"])
    if mask.any():
        return _numpy_reference(
            np.asarray(inputs["tensor_in"]), mask,
            np.asarray(inputs["Wq"]), np.asarray(inputs["Wk"]),
            np.asarray(inputs["Wv"]), np.asarray(inputs["Wo"]),
        )
    out, _ = _run(inputs, trace=False)
    return out



# revision 7
# speedup vs baseline: 1.1702x; 1.1702x over previous
"""Multi-head attention (B=2, S=2048, D=1024, H=16) on 8 TRN2 NeuronCores.

Sharding: tensor parallel over heads (2 heads/core) for QKV projection +
attention, then 4 chunked AllToAlls of the context (channel-shard ->
row-shard), then row-parallel output projection. Inputs arrive full;
sharding happens host-side in `kernel()`.

Design notes (v2):
- Scores matmuls are row-tiled: head0 contracts on PE rows 0-63, head1 on
  rows 64-127, so both heads' score matmuls stream concurrently (the PE
  runs 32x32 subarrays independently) -- scores cost ~halves vs padded
  128-contraction per head.
- Both heads' scores land in one [128, 2, 512] PSUM tile so a single
  wide ACT instruction computes exp for both heads (fewer ACT overheads;
  ACT is the critical engine at ~147us of exp work).
- Softmax normalization is deferred past the AllToAll: we ship the
  unnormalized context plus the exp-sums row (65 rows/head) and divide
  on the receiving side, where the reciprocal is a wide cheap op instead
  of a [1, N] single-partition DVE reciprocal on the critical path.
- The A2A is split into 4 chunks (one per 512-row q-slab) so the last
  collective carries only 1/4 of the payload and the output projection
  pipelines behind the earlier chunks.
"""

import numpy as np

B, S, D, H = 2, 2048, 1024, 16
NCORES = 8
CH = D // NCORES          # 128 channels (2 heads) per core
HD = D // H               # 64
ROWS = B * S              # 4096
RPC = ROWS // NCORES      # 512 rows per core for the output projection
KO = D // 128             # 8 contraction chunks of 128
QCH = 512                 # q-chunk processed per attention pass
NQ = S // QCH             # 4 passes per batch
KB = S // 128             # 16 key blocks
RH = RPC // NQ            # 128 rows per core per A2A chunk
SCALE = 1.0 / 32.0        # 1/sqrt(D)

# Row-tiled scores (64-row PE tiles, both heads concurrent) vs padded-q
# 128-contraction scores (baseline style; PE stays in 128x128 mode).
ROWTILE_SCORES = True

_CACHE = {}


def _build():
    import concourse.mybir as mybir
    import concourse.tile as tile
    from concourse import bacc
    from concourse.masks import make_identity

    BF16 = mybir.dt.bfloat16
    F32 = mybir.dt.float32
    AF = mybir.ActivationFunctionType

    nc = bacc.Bacc("TRN2", target_bir_lowering=False, debug=False, num_devices=NCORES)
    xT = nc.dram_tensor("xT", [D, ROWS], BF16, kind="ExternalInput")
    # weights arrive host-pre-tiled as [128, KO, out] so DMAs are contiguous
    wq = nc.dram_tensor("wq", [128, KO, CH], BF16, kind="ExternalInput")
    wk = nc.dram_tensor("wk", [128, KO, CH], BF16, kind="ExternalInput")
    wv = nc.dram_tensor("wv", [128, KO, CH], BF16, kind="ExternalInput")
    wo = nc.dram_tensor("wo", [128, KO, D], BF16, kind="ExternalInput")
    out = nc.dram_tensor("out", [RPC, D], F32, kind="ExternalOutput")

    NRB = S // 512  # rowblocks per batch (= NQ)

    with tile.TileContext(nc) as tc:
        with (
            tc.tile_pool(name="const", bufs=1) as cpool,
            tc.tile_pool(name="kv", bufs=16) as kvp,
            tc.tile_pool(name="vt", bufs=3) as vtp,
            tc.tile_pool(name="vtr", bufs=8) as vtrp,
            tc.tile_pool(name="xt", bufs=6) as xtp,
            tc.tile_pool(name="exp", bufs=6) as expp,
            tc.tile_pool(name="cs", bufs=4) as csp,
            tc.tile_pool(name="ph2", bufs=2) as ph2p,
            tc.tile_pool(name="nrm", bufs=2) as nrmp,
            tc.tile_pool(name="osb", bufs=2) as osbp,
            tc.tile_pool(name="ps", bufs=2, space="PSUM") as ps,
            tc.tile_pool(name="dram", bufs=1, space="DRAM") as dram,
        ):
            w_tiles = {}
            for name, t in (("wq", wq), ("wk", wk), ("wv", wv)):
                wt = cpool.tile([128, KO, CH], BF16, tag=name)
                nc.sync.dma_start(wt[:], t[:])
                w_tiles[name] = wt
            ident = cpool.tile([128, 128], BF16, tag="ident")
            make_identity(nc, ident[:])

            a2a_in = [dram.tile([NCORES, 130, RH], BF16, name=f"a2a_in{p}") for p in range(NQ)]
            a2a_out = [dram.tile([NCORES, 130, RH], BF16, name=f"a2a_out{p}") for p in range(NQ)]

            xT_r = xT.ap().rearrange("(ko p) n -> p ko n", p=128)

            qts = {0: [None] * NRB, 1: [None] * NRB}
            kts = {0: [None] * NRB, 1: [None] * NRB}
            vrs = {0: [None] * NRB, 1: [None] * NRB}

            def proj_rowblock(b, rb):
                """project one 512-row block; V transposed into row-major
                [krows, ch] blocks with a fused ones column per head."""
                r = b * NRB + rb
                xt = xtp.tile([128, KO, 512], BF16, tag="xt")
                nc.sync.dma_start(xt[:], xT_r[:, :, r * 512:(r + 1) * 512])
                qt = kvp.tile([128, 512], BF16, tag="qt", name=f"qt{b}_{rb}")
                kt = kvp.tile([128, 512], BF16, tag="kt", name=f"kt{b}_{rb}")
                vt = vtp.tile([128, 512], BF16, tag="vt")
                for wname, dst in (("wq", qt), ("wk", kt), ("wv", vt)):
                    pj = ps.tile([128, 512], F32, tag="pj")
                    for ko in range(KO):
                        nc.tensor.matmul(
                            pj[:], w_tiles[wname][:, ko, :], xt[:, ko, :],
                            start=(ko == 0), stop=(ko == KO - 1),
                        )
                    nc.vector.tensor_copy(dst[:], pj[:])
                # vr: per head a full-128 lhsT block [V_h | 1 | zeros]
                vr = vtrp.tile([128, 4, 256], BF16, tag="vtr", name=f"vr{b}_{rb}")
                nc.gpsimd.memset(vr[:], 0.0)
                nc.gpsimd.memset(vr[:, :, 64:65], 1.0)
                nc.gpsimd.memset(vr[:, :, 192:193], 1.0)
                for j in range(4):
                    tp = ps.tile([128, 128], BF16, tag="pj", name=f"tp{b}_{rb}_{j}")
                    nc.tensor.transpose(tp[:], vt[:, j * 128:(j + 1) * 128], ident[:])
                    nc.vector.tensor_copy(vr[:, j, 0:64], tp[:, 0:64])
                    nc.vector.tensor_copy(vr[:, j, 128:192], tp[:, 64:128])
                qts[b][rb], kts[b][rb], vrs[b][rb] = qt, kt, vr

            def attn_chunk(b, p, ctx_ps, kb_range):
                for kb in kb_range:
                    krb, kj = kb // 4, kb % 4
                    # both heads' scores in one 2-bank psum tile; the two
                    # matmuls are row-tiled (h0 rows 0-63, h1 rows 64-127)
                    # and stream concurrently through the PE array
                    sc = ps.tile([128, 2, 512], F32, tag="sc", name=f"sc_{b}_{p}_{kb}")
                    for h in range(2):
                        nc.tensor.matmul(
                            sc[:, h, :],
                            kts[b][krb][h * 64:(h + 1) * 64, kj * 128:(kj + 1) * 128],
                            qts[b][p][h * 64:(h + 1) * 64, :],
                            start=True, stop=True,
                        )
                    ex = expp.tile([128, 2, 512], BF16, tag="exp")
                    nc.scalar.activation(
                        ex[:].rearrange("p h n -> p (h n)"),
                        sc[:].rearrange("p h n -> p (h n)"),
                        AF.Exp, scale=SCALE,
                    )
                    for h in range(2):
                        nc.tensor.matmul(
                            ctx_ps[h][:],
                            vrs[b][krb][:, kj, h * 128:(h + 1) * 128],
                            ex[:, h, :],
                            start=(kb == 0), stop=(kb == KB - 1),
                        )

            def ship(b, p, ctx_ps):
                # unnormalized context + exp-sum row (65 rows/head), cast to
                # bf16 straight from PSUM and scattered into the A2A buffer;
                # q within the chunk decomposes as (j, i) -> dst core 4b+j,
                # local row p*128+i
                for h in range(2):
                    cf = csp.tile([65, 512], BF16, tag="cs", name=f"cs_{b}_{p}_{h}")
                    nc.vector.tensor_copy(cf[:], ctx_ps[h][0:65, :])
                    nc.sync.dma_start(
                        a2a_in[p][4 * b:4 * b + 4, h * 65:(h + 1) * 65, :]
                        .rearrange("j c i -> c j i"),
                        cf[:].rearrange("c (j i) -> c j i", i=RH),
                    )

            def phase2(p, wo_t):
                # gather this chunk's full-channel context: channel d =
                # j*128 + (h*64+c) where j is the src core; strip sum rows
                ctxg = ph2p.tile([128, KO, RH], BF16, tag="ctxg", name=f"ctxg{p}")
                ctxn = ph2p.tile([128, KO, RH], BF16, tag="ctxn", name=f"ctxn{p}")
                scl = nrmp.tile([128, KO * RH], F32, tag="scl", name=f"scl{p}")
                for h in range(2):
                    nc.sync.dma_start(
                        ctxg[h * 64:(h + 1) * 64, :, :],
                        a2a_out[p][:, h * 65:h * 65 + 64, :]
                        .rearrange("j c i -> c j i"),
                    )
                    # sum row for this head-parity, laid out (j, i); the
                    # reciprocal is a cheap wide DVE op on this side
                    smb = nrmp.tile([1, KO * RH], BF16, tag=f"smb{h}", name=f"smb{p}_{h}")
                    nc.sync.dma_start(
                        smb[:].rearrange("c (j i) -> c j i", j=KO),
                        a2a_out[p][:, h * 65 + 64:h * 65 + 65, :]
                        .rearrange("j c i -> c j i"),
                    )
                    smf = nrmp.tile([1, KO * RH], F32, tag=f"smf{h}", name=f"smf{p}_{h}")
                    nc.vector.tensor_copy(smf[:], smb[:])
                    rc = nrmp.tile([1, KO * RH], F32, tag=f"rc{h}", name=f"rc{p}_{h}")
                    nc.vector.reciprocal_approx_fast(rc[:], smf[:])
                    if h == 0:
                        nc.gpsimd.partition_broadcast(
                            scl[0:64, :], rc[:], channels=64)
                    else:
                        # partition_broadcast to a base-0 tile, then DVE-copy
                        # into the upper half (broadcast dst base!=0 is
                        # unproven on hw; DVE copies to base-64 dst are not)
                        sch = nrmp.tile([64, KO * RH], F32, tag="sch", name=f"sch{p}")
                        nc.gpsimd.partition_broadcast(sch[:], rc[:], channels=64)
                        nc.vector.tensor_copy(scl[64:128, :], sch[:])
                nc.vector.tensor_mul(
                    ctxn[:].rearrange("p j i -> p (j i)"),
                    ctxg[:].rearrange("p j i -> p (j i)"),
                    scl[:],
                )
                for nh in range(D // 512):
                    pj = ps.tile([128, 512], F32, tag="pj", name=f"p2_{p}_{nh}")
                    for j in range(KO):
                        nc.tensor.matmul(
                            pj[:],
                            ctxn[:, j, :],
                            wo_t[:, j, nh * 512:(nh + 1) * 512],
                            start=(j == 0), stop=(j == KO - 1),
                        )
                    ob = osbp.tile([128, 512], F32, tag="osb")
                    nc.vector.tensor_copy(ob[:], pj[:])
                    nc.sync.dma_start(
                        out.ap()[p * RH:(p + 1) * RH, nh * 512:(nh + 1) * 512],
                        ob[:],
                    )

            def ctx_alloc(b, p):
                return [
                    ps.tile([128, 512], F32, tag="cx", name=f"ctx_{b}_{p}_{h}")
                    for h in range(2)
                ]

            def attn_pass(b, p, interleave=()):
                ctx = ctx_alloc(b, p)
                il = list(interleave)
                for g in range(4):
                    if g < len(il):
                        il[g]()
                    attn_chunk(b, p, ctx, range(g * 4, (g + 1) * 4))
                ship(b, p, ctx)

            # b0: proj pipelined inside the first attention passes so the
            # ScalarE starts on exp as early as possible
            proj_rowblock(0, 0)
            attn_pass(0, 0, (
                lambda: proj_rowblock(0, 1),
                lambda: proj_rowblock(0, 2),
                lambda: proj_rowblock(0, 3),
            ))
            attn_pass(0, 1, (
                lambda: proj_rowblock(1, 0),
                lambda: proj_rowblock(1, 1),
            ))
            attn_pass(0, 2, (
                lambda: proj_rowblock(1, 2),
                lambda: proj_rowblock(1, 3),
            ))
            wo_t = cpool.tile([128, KO, D], BF16, tag="wo")
            nc.sync.dma_start(wo_t[:], wo[:])
            attn_pass(0, 3)
            for p in range(NQ):
                attn_pass(1, p)
                # phase2(p-1) must be emitted BEFORE collective #p: Tile
                # orders post-collective work after the collective, and
                # phase2(p-1) only depends on A2A #(p-1)'s output
                if p > 0:
                    phase2(p - 1, wo_t)
                nc.gpsimd.collective_compute(
                    "AllToAll", mybir.AluOpType.bypass,
                    replica_groups=[list(range(NCORES))],
                    ins=[a2a_in[p].opt()], outs=[a2a_out[p].opt()],
                )
            phase2(NQ - 1, wo_t)
    nc.compile()
    return nc


def _numpy_reference(tensor_in, attention_mask, Wq, Wk, Wv, Wo):
    """Fallback for a non-zero mask (never hit with the spec's zero mask)."""
    x = tensor_in.astype(np.float64)
    q = (x @ Wq.T.astype(np.float64)).reshape(B, S, H, HD).transpose(0, 2, 1, 3)
    k = (x @ Wk.T.astype(np.float64)).reshape(B, S, H, HD).transpose(0, 2, 1, 3)
    v = (x @ Wv.T.astype(np.float64)).reshape(B, S, H, HD).transpose(0, 2, 1, 3)
    scores = np.einsum("bhqd,bhkd->bhqk", q, k) + attention_mask.astype(np.float64)
    scores = scores / np.sqrt(D)
    scores -= scores.max(axis=-1, keepdims=True)
    w = np.exp(scores)
    w /= w.sum(axis=-1, keepdims=True)
    ctx = np.einsum("bhqk,bhkd->bhqd", w, v).transpose(0, 2, 1, 3).reshape(B, S, D)
    return (ctx @ Wo.T.astype(np.float64)).astype(np.float32)


def _pretile(wT: np.ndarray) -> np.ndarray:
    """[D, M] -> [128, KO, M] with row d = ko*128 + p."""
    m = wT.shape[1]
    return np.ascontiguousarray(wT.reshape(KO, 128, m).transpose(1, 0, 2))


def _row_map() -> np.ndarray:
    """global row index handled by (core c, local row lr)."""
    m = np.empty((NCORES, RPC), dtype=np.int64)
    for c in range(NCORES):
        bb, jj = c // 4, c % 4
        for p in range(NQ):
            g = bb * S + p * 512 + jj * 128
            m[c, p * RH:(p + 1) * RH] = np.arange(g, g + RH)
    return m


def _run(inputs, trace=False):
    import ml_dtypes
    from concourse.bass_utils import run_bass_kernel_spmd

    bf16 = ml_dtypes.bfloat16
    tensor_in = np.asarray(inputs["tensor_in"], dtype=np.float32)
    Wq = np.asarray(inputs["Wq"], dtype=np.float32)
    Wk = np.asarray(inputs["Wk"], dtype=np.float32)
    Wv = np.asarray(inputs["Wv"], dtype=np.float32)
    Wo = np.asarray(inputs["Wo"], dtype=np.float32)

    xT = np.ascontiguousarray(tensor_in.reshape(ROWS, D).T).astype(bf16)
    wqT = Wq.T.astype(bf16)
    wkT = Wk.T.astype(bf16)
    wvT = Wv.T.astype(bf16)
    wo_p = _pretile(Wo.T.astype(bf16))

    in_maps = []
    for c in range(NCORES):
        sl = slice(c * CH, (c + 1) * CH)
        in_maps.append({
            "xT": xT,
            "wq": _pretile(wqT[:, sl]),
            "wk": _pretile(wkT[:, sl]),
            "wv": _pretile(wvT[:, sl]),
            "wo": wo_p,
        })

    if "nc" not in _CACHE:
        _CACHE["nc"] = _build()
    res = run_bass_kernel_spmd(
        _CACHE["nc"], in_maps, core_ids=list(range(NCORES)), trace=trace
    )
    rm = _CACHE.setdefault("rm", _row_map())
    full = np.empty((ROWS, D), dtype=np.float32)
    for c in range(NCORES):
        full[rm[c]] = res.results[c]["out"]
    return full.reshape(B, S, D), res


def kernel(**inputs) -> np.ndarray:
    mask = np.asarray(inputs["attention_mask"])
    if mask.any():
        return _numpy_reference(
            np.asarray(inputs["tensor_in"]), mask,
            np.asarray(inputs["Wq"]), np.asarray(inputs["Wk"]),
            np.asarray(inputs["Wv"]), np.asarray(inputs["Wo"]),
        )
    out, _ = _run(inputs, trace=False)
    return out
